# revision 1
# baseline (speedup 1.0000x reference)
# CIN (Compressed Interaction Network) Trainium2 Bass kernel.
#
# Reference computation (B=512, F0=40, D=32, sizes=[200,200,200]):
#   h0 = x                                  [B, 40, 32]
#   per layer l: z[b,(i,j),d] = x[b,i,d] * h[b,j,d];  h' = z^T W_l  [B, 200, 32]
#   out = concat(h1,h2,h3, axis=1).sum(-1)  [B, 600]
#
# Strategy: data-parallel over batch across 8 cores (64 batches/core).
# Per core, columns c = b_local*32 + d (C=2048), processed in 4 tiles of 512.
# All tensors are kept in a [row, c] layout so each layer's GEMM output
# (h'[n, c]) is directly consumable as the next layer's h[j, c] input:
#   zT[(i,j), c] = x0T[i, c] * hT[j, c]
#   h'[n, c]     = sum_k W[k, n] * zT[k, c]   (PE, contraction over partitions)
# The x0T[i,:] partition-broadcast tiles are produced by K=40 selector
# matmuls on the PE (one per i), converted to bf16 by the scalar engine, and
# the elementwise multiply runs on the vector engine in bf16 (2x mode).
# Contraction is chunked per-i as K=128 (j 0..127) + K=72 (j 128..199) so the
# vector-engine operands stay partition-aligned with the hT tiles.

import numpy as np
import ml_dtypes

B, F0, D, S = 512, 40, 32, 200
NCORES = 8
BPC = B // NCORES          # 64 batches per core
C = BPC * D                # 2048 columns per core
CT = 512                   # column tile
NCT = C // CT              # 4 column tiles
L0K = 14                   # layer-0 chunks of 120 rows (last chunk zero-padded)

bf16 = ml_dtypes.bfloat16

_CACHE = {}


def _build_nc():
    from contextlib import ExitStack
    import concourse.tile as tile
    from concourse import bacc, mybir

    nc = bacc.Bacc("TRN2", target_bir_lowering=False, debug=False,
                   num_devices=NCORES)

    dt = mybir.dt
    ein, eout = "ExternalInput", "ExternalOutput"
    x0T_d = nc.dram_tensor("x0T", [F0, C], dt.bfloat16, kind=ein).ap()
    x0T3_d = nc.dram_tensor("x0T3", [120, C], dt.bfloat16, kind=ein).ap()
    sel_d = nc.dram_tensor("sel", [F0, F0 * 128], dt.bfloat16, kind=ein).ap()
    t0sel_d = nc.dram_tensor("t0sel", [F0, L0K * 120], dt.bfloat16, kind=ein).ap()
    w0_d = nc.dram_tensor("w0", [120, L0K * S], dt.bfloat16, kind=ein).ap()
    w1a_d = nc.dram_tensor("w1a", [128, F0 * S], dt.bfloat16, kind=ein).ap()
    w1b_d = nc.dram_tensor("w1b", [72, F0 * S], dt.bfloat16, kind=ein).ap()
    w2a_d = nc.dram_tensor("w2a", [128, F0 * S], dt.bfloat16, kind=ein).ap()
    w2b_d = nc.dram_tensor("w2b", [72, F0 * S], dt.bfloat16, kind=ein).ap()
    ident_d = nc.dram_tensor("ident", [128, 128], dt.float32, kind=ein).ap()
    y_d = nc.dram_tensor("y", [BPC, 3 * S], dt.float32, kind=eout).ap()

    with tile.TileContext(nc) as tc, ExitStack() as ctx:
        const = ctx.enter_context(tc.tile_pool(name="const", bufs=1))
        xpool = ctx.enter_context(tc.tile_pool(name="xpool", bufs=1))
        zpool = ctx.enter_context(tc.tile_pool(name="zpool", bufs=4))
        hpool = ctx.enter_context(tc.tile_pool(name="hpool", bufs=2))
        ps = ctx.enter_context(tc.tile_pool(name="ps", bufs=2, space="PSUM"))

        def const_tile(name, shape, dtp, src):
            t = const.tile(shape, dtp, name=name, tag=name)
            nc.sync.dma_start(t[:], src[:])
            return t

        x0T = const_tile("x0T_sb", [F0, C], dt.bfloat16, x0T_d)
        x0T3 = const_tile("x0T3_sb", [120, C], dt.bfloat16, x0T3_d)
        sel = const_tile("sel_sb", [F0, F0 * 128], dt.bfloat16, sel_d)
        t0sel = const_tile("t0sel_sb", [F0, L0K * 120], dt.bfloat16, t0sel_d)
        w0 = const_tile("w0_sb", [120, L0K * S], dt.bfloat16, w0_d)
        w1a = const_tile("w1a_sb", [128, F0 * S], dt.bfloat16, w1a_d)
        w1b = const_tile("w1b_sb", [72, F0 * S], dt.bfloat16, w1b_d)
        w2a = const_tile("w2a_sb", [128, F0 * S], dt.bfloat16, w2a_d)
        w2b = const_tile("w2b_sb", [72, F0 * S], dt.bfloat16, w2b_d)
        ident = const_tile("ident_sb", [128, 128], dt.float32, ident_d)

        # per-layer output accumulators [n, b_local] in fp32
        outA = [const.tile([128, BPC], dt.float32, name=f"outA{l}", tag=f"outA{l}")
                for l in range(3)]
        outB = [const.tile([72, BPC], dt.float32, name=f"outB{l}", tag=f"outB{l}")
                for l in range(3)]
        final = const.tile([BPC, 3 * S], dt.float32, name="final", tag="final")

        for ct in range(NCT):
            c0 = ct * CT
            csl = slice(c0, c0 + CT)

            # ---- build x0 broadcast tiles (shared by all 3 layers) ----
            xt = []
            for i in range(F0):
                xp = ps.tile([128, CT], dt.float32, name=f"xp{i}", tag="xps")
                nc.tensor.matmul(xp[:], sel[:, i * 128:(i + 1) * 128],
                                 x0T[:, csl], start=True, stop=True)
                xi = xpool.tile([128, CT], dt.bfloat16, name=f"x{i}",
                                tag=f"x{i}")
                nc.scalar.copy(xi[:], xp[:])
                xt.append(xi)

            # ---- layer 0 (h = x0, interaction rows chunked by 120) ----
            accA = ps.tile([128, CT], dt.float32, name="accA0", tag="accA")
            accB = ps.tile([72, CT], dt.float32, name="accB0", tag="accB")
            for k in range(L0K):
                xp0 = ps.tile([128, CT], dt.float32, name=f"xp0{k}", tag="xps")
                nc.tensor.matmul(xp0[0:120, :], t0sel[:, k * 120:(k + 1) * 120],
                                 x0T[:, csl], start=True, stop=True)
                x0p = xpool.tile([120, CT], dt.bfloat16, name=f"x0p{k}",
                                 tag="x0p", bufs=4)
                nc.scalar.copy(x0p[:], xp0[0:120, :])
                z0 = zpool.tile([120, CT], dt.bfloat16, name=f"z0{k}", tag="z0")
                nc.vector.tensor_mul(z0[:], x0T3[:, csl], x0p[:])
                nc.tensor.matmul(accA[:], w0[:, k * S:k * S + 128], z0[:],
                                 start=(k == 0), stop=(k == L0K - 1))
                nc.tensor.matmul(accB[:], w0[:, k * S + 128:(k + 1) * S], z0[:],
                                 start=(k == 0), stop=(k == L0K - 1))

            # ---- layers 1, 2 ----
            for l, (wa, wb) in enumerate(((w1a, w1b), (w2a, w2b)), start=1):
                # finish previous layer: d-sums + (bf16 h for next layer)
                nc.vector.tensor_reduce(
                    outA[l - 1][:, ct * 16:(ct + 1) * 16],
                    accA[:].rearrange("p (b d) -> p b d", d=D),
                    axis=mybir.AxisListType.X, op=mybir.AluOpType.add)
                nc.vector.tensor_reduce(
                    outB[l - 1][:, ct * 16:(ct + 1) * 16],
                    accB[:].rearrange("p (b d) -> p b d", d=D),
                    axis=mybir.AxisListType.X, op=mybir.AluOpType.add)
                hA = hpool.tile([128, CT], dt.bfloat16, name=f"hA{l}", tag="hA")
                hB = hpool.tile([72, CT], dt.bfloat16, name=f"hB{l}", tag="hB")
                nc.scalar.copy(hA[:], accA[:])
                nc.scalar.copy(hB[:], accB[:])

                accA = ps.tile([128, CT], dt.float32, name=f"accA{l}", tag="accA")
                accB = ps.tile([72, CT], dt.float32, name=f"accB{l}", tag="accB")
                for i in range(F0):
                    za = zpool.tile([128, CT], dt.bfloat16, name=f"za{l}_{i}",
                                    tag="za")
                    nc.vector.tensor_mul(za[:], hA[:], xt[i][:])
                    zb = zpool.tile([72, CT], dt.bfloat16, name=f"zb{l}_{i}",
                                    tag="zb")
                    nc.vector.tensor_mul(zb[:], hB[:], xt[i][0:72, :])
                    st, sp = (i == 0), (i == F0 - 1)
                    nc.tensor.matmul(accA[:], wa[:, i * S:i * S + 128], za[:],
                                     start=st, stop=False)
                    nc.tensor.matmul(accB[:], wa[:, i * S + 128:(i + 1) * S],
                                     za[:], start=st, stop=False)
                    nc.tensor.matmul(accA[:], wb[:, i * S:i * S + 128], zb[:],
                                     start=False, stop=sp)
                    nc.tensor.matmul(accB[:], wb[:, i * S + 128:(i + 1) * S],
                                     zb[:], start=False, stop=sp)

            # final layer d-sums
            nc.vector.tensor_reduce(
                outA[2][:, ct * 16:(ct + 1) * 16],
                accA[:].rearrange("p (b d) -> p b d", d=D),
                axis=mybir.AxisListType.X, op=mybir.AluOpType.add)
            nc.vector.tensor_reduce(
                outB[2][:, ct * 16:(ct + 1) * 16],
                accB[:].rearrange("p (b d) -> p b d", d=D),
                axis=mybir.AxisListType.X, op=mybir.AluOpType.add)

        # ---- transpose [n, b] -> [b, n] and assemble final [64, 600] ----
        for l in range(3):
            tpA = ps.tile([BPC, 128], dt.float32, name=f"tpA{l}", tag="tps")
            nc.tensor.transpose(tpA[:], outA[l][:], ident[:])
            nc.scalar.copy(final[:, l * S:l * S + 128], tpA[:])
            tpB = ps.tile([BPC, 72], dt.float32, name=f"tpB{l}", tag="tps")
            nc.tensor.transpose(tpB[:], outB[l][:], ident[0:72, 0:72])
            nc.scalar.copy(final[:, l * S + 128:(l + 1) * S], tpB[:])

        nc.sync.dma_start(y_d[:], final[:])

    nc.compile()
    return nc


def _prep_consts(W0, W1, W2):
    """Host-side constant tensors shared by all cores (bf16)."""
    W0b = W0.astype(bf16)
    W1b = W1.astype(bf16)
    W2b = W2.astype(bf16)

    # layer-0 weights: chunks of 120 interaction rows, zero-padded to 14*120
    w0p = np.zeros((120, L0K * S), dtype=bf16)
    for k in range(L0K):
        r0 = k * 120
        rows = min(120, F0 * F0 - r0)
        w0p[:rows, k * S:(k + 1) * S] = W0b[r0:r0 + rows]

    # layer 1/2 weights: per-i chunks, j 0..127 (A) and j 128..199 (B)
    def wsplit(Wb):
        wa = np.zeros((128, F0 * S), dtype=bf16)
        wb = np.zeros((72, F0 * S), dtype=bf16)
        for i in range(F0):
            wa[:, i * S:(i + 1) * S] = Wb[i * S:i * S + 128]
            wb[:, i * S:(i + 1) * S] = Wb[i * S + 128:(i + 1) * S]
        return wa, wb

    w1a, w1b = wsplit(W1b)
    w2a, w2b = wsplit(W2b)

    # selector for broadcasting x0T row i across 128 partitions
    sel = np.zeros((F0, F0 * 128), dtype=bf16)
    for i in range(F0):
        sel[i, i * 128:(i + 1) * 128] = 1.0

    # selector for the layer-0 x0 broadcast pattern (3 i-blocks per chunk)
    t0sel = np.zeros((F0, L0K * 120), dtype=bf16)
    for k in range(L0K):
        for r in range(120):
            i = 3 * k + r // F0
            if i < F0:
                t0sel[i, k * 120 + r] = 1.0

    ident = np.eye(128, dtype=np.float32)
    return dict(sel=sel, t0sel=t0sel, w0=w0p, w1a=w1a, w1b=w1b,
                w2a=w2a, w2b=w2b, ident=ident)


def _prep_in_maps(inputs, W0, W1, W2):
    consts = _prep_consts(np.asarray(W0, np.float32),
                          np.asarray(W1, np.float32),
                          np.asarray(W2, np.float32))
    x = np.asarray(inputs, np.float32)
    in_maps = []
    for c in range(NCORES):
        xs = x[c * BPC:(c + 1) * BPC]                    # [64, 40, 32]
        x0T = np.ascontiguousarray(
            xs.transpose(1, 0, 2).reshape(F0, C)).astype(bf16)
        x0T3 = np.ascontiguousarray(np.tile(x0T, (3, 1)))  # [120, C]
        in_maps.append(dict(x0T=x0T, x0T3=x0T3, **consts))
    return in_maps


def _get_nc():
    if "nc" not in _CACHE:
        _CACHE["nc"] = _build_nc()
    return _CACHE["nc"]


def kernel(inputs, W0, W1, W2):
    from concourse.bass_utils import run_bass_kernel_spmd

    nc = _get_nc()
    in_maps = _prep_in_maps(inputs, W0, W1, W2)
    res = run_bass_kernel_spmd(nc, in_maps, core_ids=list(range(NCORES)))
    y = np.concatenate([res.results[c]["y"] for c in range(NCORES)], axis=0)
    return np.ascontiguousarray(y, dtype=np.float32)


# revision 40
# speedup vs baseline: 2285.5259x; 2285.5259x over previous
# CIN (Compressed Interaction Network) Trainium2 Bass kernel.
#
# Reference computation (B=512, F0=40, D=32, sizes=[200,200,200]):
#   h0 = x                                  [B, 40, 32]
#   per layer l: z[b,(i,j),d] = x[b,i,d] * h[b,j,d];  h' = z^T W_l  [B, 200, 32]
#   out = concat(h1,h2,h3, axis=1).sum(-1)  [B, 600]
#
# Strategy: data-parallel over batch across 8 cores (64 batches/core).
# Per core, columns c = b_local*32 + d (C=2048), processed in 4 tiles of 512.
# All tensors are kept in a [row, c] layout so each layer's GEMM output
# (h'[n, c]) is directly consumable as the next layer's h[j, c] input:
#   zT[(i,j), c] = x0T[i, c] * hT[j, c]
#   h'[n, c]     = sum_k W[k, n] * zT[k, c]   (PE, contraction over partitions)
# The x0T[i,:] partition-broadcast tiles are produced by K=40 selector
# matmuls on the PE (one per i), converted to bf16 by the scalar engine, and
# the elementwise multiply runs on the vector engine in bf16 (2x mode).
# Contraction is chunked per-i as K=128 (j 0..127) + K=72 (j 128..199) so the
# vector-engine operands stay partition-aligned with the hT tiles.

import dataclasses

import numpy as np
import ml_dtypes

B, F0, D, S = 512, 40, 32, 200
NCORES = 8
BPC = B // NCORES          # 64 batches per core
C = BPC * D                # 2048 columns per core
CT = 512                   # column tile
NCT = C // CT              # 4 column tiles
L0K = 14                   # layer-0 chunks of 120 rows (last chunk zero-padded)

bf16 = ml_dtypes.bfloat16

# production configuration
XMODE, GB, TTW = "dma", 8, 4

_CACHE = {}


def _build_nc(repeat=1, skip=(), xmode=XMODE, gb=GB, ttw=TTW):
    """skip: ablation variants for timing. xmode: "pe" (selector matmuls) or
    "dma" (doubling chains) for the x0 broadcast tiles. gb: i-block size for
    PSUM-bank-grouped matmul emission."""
    from contextlib import ExitStack
    import concourse.tile as tile
    from concourse import bacc, mybir

    nsl = slice(0, 4) if "mm" in skip else slice(0, None)

    nc = bacc.Bacc("TRN2", target_bir_lowering=False, debug=False,
                   num_devices=NCORES)

    dt = mybir.dt
    ein, eout = "ExternalInput", "ExternalOutput"
    x0T_d = nc.dram_tensor("x0T", [F0, C], dt.bfloat16, kind=ein).ap()
    x0T3_d = nc.dram_tensor("x0T3", [120, C], dt.bfloat16, kind=ein).ap()
    w0_d = nc.dram_tensor("w0", [120, L0K * S], dt.bfloat16, kind=ein).ap()
    t0sel_d = nc.dram_tensor("t0sel", [F0, L0K * 120], dt.bfloat16,
                             kind=ein).ap()
    if xmode == "pe":
        sel_d = nc.dram_tensor("sel", [F0, F0 * 128], dt.bfloat16,
                               kind=ein).ap()
    w1a_d = nc.dram_tensor("w1a", [128, F0 * S], dt.bfloat16, kind=ein).ap()
    w1b_d = nc.dram_tensor("w1b", [72, F0 * S], dt.bfloat16, kind=ein).ap()
    w2a_d = nc.dram_tensor("w2a", [128, F0 * S], dt.bfloat16, kind=ein).ap()
    w2b_d = nc.dram_tensor("w2b", [72, F0 * S], dt.bfloat16, kind=ein).ap()
    ident_d = nc.dram_tensor("ident", [128, 128], dt.float32, kind=ein).ap()
    y_d = nc.dram_tensor("y", [BPC, 3 * S], dt.float32, kind=eout).ap()

    with tile.TileContext(nc) as tc, ExitStack() as ctx:
        const = ctx.enter_context(tc.tile_pool(name="const", bufs=1))
        xpool = ctx.enter_context(tc.tile_pool(name="xpool", bufs=1))
        zpool = ctx.enter_context(tc.tile_pool(name="zpool", bufs=4))
        hpool = ctx.enter_context(tc.tile_pool(name="hpool", bufs=2))
        ps = ctx.enter_context(tc.tile_pool(name="ps", bufs=2, space="PSUM"))

        def const_tile(name, shape, dtp, src):
            t = const.tile(shape, dtp, name=name, tag=name)
            nc.sync.dma_start(t[:], src[:])
            return t

        x0T = const_tile("x0T_sb", [F0, C], dt.bfloat16, x0T_d)
        x0T3 = const_tile("x0T3_sb", [120, C], dt.bfloat16, x0T3_d)
        w0 = const_tile("w0_sb", [120, L0K * S], dt.bfloat16, w0_d)
        t0sel = const_tile("t0sel_sb", [F0, L0K * 120], dt.bfloat16, t0sel_d)
        if xmode == "pe":
            sel = const_tile("sel_sb", [F0, F0 * 128], dt.bfloat16, sel_d)
        # w1/w2 tiles are allocated now but their load DMAs are issued after
        # the first layer-0 chunk so they don't head-block the HWDGE ring
        w1a = const.tile([128, F0 * S], dt.bfloat16, name="w1a_sb", tag="w1a_sb")
        w1b = const.tile([72, F0 * S], dt.bfloat16, name="w1b_sb", tag="w1b_sb")
        w2a = const.tile([128, F0 * S], dt.bfloat16, name="w2a_sb", tag="w2a_sb")
        w2b = const.tile([72, F0 * S], dt.bfloat16, name="w2b_sb", tag="w2b_sb")
        ident = const_tile("ident_sb", [128, 128], dt.float32, ident_d)
        wloads = [(w1a, w1a_d), (w1b, w1b_d), (w2a, w2a_d), (w2b, w2b_d)]

        # per-layer output accumulators [n, b_local] in fp32
        outA = [const.tile([128, BPC], dt.float32, name=f"outA{l}", tag=f"outA{l}")
                for l in range(3)]
        outB = [const.tile([72, BPC], dt.float32, name=f"outB{l}", tag=f"outB{l}")
                for l in range(3)]
        final = const.tile([BPC, 3 * S], dt.float32, name="final", tag="final")

        rep_ctx = tc.For_i(0, repeat, 1) if repeat > 1 else None
        if rep_ctx is not None:
            rep_ctx.__enter__()

        HI = F0 // 2
        zsl = slice(0, 4) if "tt" in skip else slice(0, CT)
        st8 = {}   # per-ct pipeline state: xfs, xts, acc

        def emit_xf(ct):
            """x0 broadcast tensors Xf[p, i*CT+c] = x0T[i, ct*CT+c] (dma)."""
            c0 = ct * CT
            csl = slice(c0, c0 + CT)
            xfs = []
            if xmode == "dma":
                for h in range(2):
                    # both chains on the SP ring: ACT only carries the x0p
                    # copies, which feed layer 0 and must not queue behind
                    # chain semaphore waits
                    eng = nc.sync
                    xf = xpool.tile([128, HI * CT], dt.bfloat16,
                                    name=f"xf{h}_{ct}", tag=f"xf{h}", bufs=2)
                    if "xf" in skip:
                        eng.dma_start(
                            xf[0:1, 0:HI].rearrange("p (i c) -> p i c", c=1),
                            x0T_d[h * HI:(h + 1) * HI, c0:c0 + 1])
                    else:
                        # seed rows 0..7 independently from DRAM (no chain
                        # deps), then 4 row-doubling DMAs: 8->16->32->64->128
                        for r in range(8):
                            eng.dma_start(
                                xf[r:r + 1, :].rearrange(
                                    "p (i c) -> p i c", c=CT),
                                x0T_d[h * HI:(h + 1) * HI, csl])
                        n = 8
                        while n < 128:
                            eng.dma_start(xf[n:2 * n, :], xf[0:n, :])
                            n *= 2
                    xfs.append(xf)
            st8[ct] = {"xfs": xfs, "xts": {}}

        def make_xt(ct, i):
            # X_i[p, c] = x0T[i, c] via K=40 selector matmul + ACT copy
            c0 = ct * CT
            xp = ps.tile([128, CT], dt.float32, name=f"xp{i}", tag="xps")
            nc.tensor.matmul(xp[:], sel[:, i * 128:(i + 1) * 128],
                             x0T[:, c0:c0 + CT], start=True, stop=True)
            xi = xpool.tile([128, CT], dt.bfloat16, name=f"x{i}", tag=f"x{i}")
            nc.scalar.copy(xi[:], xp[:])
            st8[ct]["xts"][i] = xi

        def xslice(ct, i, rows=slice(0, 128)):
            if xmode == "pe":
                return st8[ct]["xts"][i][rows, :]
            h, ii = divmod(i, HI)
            return st8[ct]["xfs"][h][rows, ii * CT:(ii + 1) * CT]

        def l0_feed(ct, k):
            """x0p broadcast pattern + z0 TT for layer-0 chunk k."""
            c0 = ct * CT
            csl = slice(c0, c0 + CT)
            x0p = xpool.tile([120, CT], dt.bfloat16, name=f"x0p{k}",
                             tag="x0p", bufs=4)
            # selector matmul feed: no DMA dependencies, pipelines PE->ACT->
            # DVE tightly (the PE cost is ~3us/ct, ACT is otherwise idle)
            xp0 = ps.tile([128, CT], dt.float32, name=f"xp0{k}", tag="tps")
            nc.tensor.matmul(xp0[0:120, :],
                             t0sel[:, k * 120:(k + 1) * 120],
                             x0T[:, csl], start=True, stop=True)
            nc.scalar.copy(x0p[:], xp0[0:120, :])
            z0 = zpool.tile([120, CT], dt.bfloat16, name=f"z0_{ct}_{k}",
                            tag="z0", bufs=11)
            nc.vector.tensor_mul(z0[:, zsl], x0T3[:, c0:c0 + zsl.stop],
                                 x0p[:, zsl])
            st8[ct].setdefault("z0s", {})[k] = z0

        def emit_l0(ct, first=False):
            """Layer 0 GEMM (feeds must have been emitted)."""
            nonlocal wloads
            accA = ps.tile([128, CT], dt.float32, name=f"l0A{ct}", tag="accA",
                           bufs=3)
            accB = ps.tile([72, CT], dt.float32, name=f"l0B{ct}", tag="accB",
                           bufs=3)
            z0s = st8[ct]["z0s"]
            for k in range(L0K):
                if k not in z0s:
                    l0_feed(ct, k)
                z0 = z0s[k]
                nc.tensor.matmul(accA[:, nsl], w0[:, k * S:k * S + 128],
                                 z0[:, nsl],
                                 start=(k == 0), stop=(k == L0K - 1))
                nc.tensor.matmul(accB[:, nsl], w0[:, k * S + 128:(k + 1) * S],
                                 z0[:, nsl],
                                 start=(k == 0), stop=(k == L0K - 1))
                if first and k in (4, 10) and wloads:
                    # stagger the big weight loads so they don't contend with
                    # the x0 broadcast chains on the DMA engines at startup
                    for wt, wd in wloads[:2]:
                        nc.sync.dma_start(wt[:], wd[:])
                    wloads = wloads[2:]
            st8[ct]["acc"] = (accA, accB)

        def bcast4(ap):
            # free-dim step-0 broadcast: [p, CT] read as [p, 4, CT]
            a = [list(d) for d in ap.ap]
            return dataclasses.replace(ap, ap=[a[0], [0, 4], a[1]])

        def drain_acc(ct, l, defer=False):
            """Produce the bf16 h tiles for the next layer (critical path),
            returning the output d-sum reduces as deferred thunks so they
            don't sit between the h copies and the next layer's first TT in
            the DVE stream."""
            accA, accB = st8[ct]["acc"]
            hA = hB = None
            if l < 2:
                hA = hpool.tile([128, CT], dt.bfloat16, name=f"hA{l}",
                                tag="hA")
                hB = hpool.tile([72, CT], dt.bfloat16, name=f"hB{l}",
                                tag="hB")
                nc.vector.tensor_copy(hA[:], accA[:])
                nc.vector.tensor_copy(hB[:], accB[:])

            def reduces():
                nc.vector.tensor_reduce(
                    outA[l][:, ct * 16:(ct + 1) * 16],
                    accA[:].rearrange("p (b d) -> p b d", d=D),
                    axis=mybir.AxisListType.X, op=mybir.AluOpType.add)
                nc.vector.tensor_reduce(
                    outB[l][:, ct * 16:(ct + 1) * 16],
                    accB[:].rearrange("p (b d) -> p b d", d=D),
                    axis=mybir.AxisListType.X, op=mybir.AluOpType.add)

            if defer:
                return hA, hB, reduces
            reduces()
            return hA, hB

        def emit_layer(ct, l, wa, wb, hA, hB, feeds=None):
            """One interaction layer: z TTs + per-i GEMM accumulation.
            feeds: optional list of thunks (next-ct layer-0 feed work) to
            interleave between i-blocks so the DVE stream stays ahead."""
            xfs = st8[ct]["xfs"]
            accA = ps.tile([128, CT], dt.float32, name=f"l{l}A{ct}",
                           tag="accA", bufs=3)
            accB = ps.tile([72, CT], dt.float32, name=f"l{l}B{ct}",
                           tag="accB", bufs=3)
            for i0 in range(0, F0, gb):
                blk = range(i0, min(i0 + gb, F0))
                zas, zbs = {}, {}
                if ttw == 4 and xmode == "dma" and "tt" not in skip:
                    for j0 in range(i0, min(i0 + gb, F0), 4):
                        h4, ii = divmod(j0, HI)
                        x4 = xfs[h4][:, ii * CT:(ii + 4) * CT]
                        za4 = zpool.tile([128, 4 * CT], dt.bfloat16,
                                         name=f"za{l}_{j0}", tag="za",
                                         bufs=3)
                        nc.vector.tensor_mul(
                            za4[:].rearrange("p (r c) -> p r c", r=4),
                            bcast4(hA[:]),
                            x4.rearrange("p (r c) -> p r c", r=4))
                        zb4 = zpool.tile([72, 4 * CT], dt.bfloat16,
                                         name=f"zb{l}_{j0}", tag="zb",
                                         bufs=3)
                        nc.vector.tensor_mul(
                            zb4[:].rearrange("p (r c) -> p r c", r=4),
                            bcast4(hB[:]),
                            x4[0:72, :].rearrange("p (r c) -> p r c", r=4))
                        for i in range(j0, j0 + 4):
                            o = (i - j0) * CT
                            zas[i] = za4[:, o:o + CT]
                            zbs[i] = zb4[:, o:o + CT]
                else:
                    for i in blk:
                        if xmode == "pe" and l == 1:
                            make_xt(ct, i)
                        za = zpool.tile([128, CT], dt.bfloat16,
                                        name=f"za{l}_{i}", tag="za",
                                        bufs=gb + 2)
                        nc.vector.tensor_mul(za[:, zsl], hA[:, zsl],
                                             xslice(ct, i)[:, zsl])
                        zb = zpool.tile([72, CT], dt.bfloat16,
                                        name=f"zb{l}_{i}", tag="zb",
                                        bufs=gb + 2)
                        nc.vector.tensor_mul(zb[:, zsl], hB[:, zsl],
                                             xslice(ct, i, slice(0, 72))[:, zsl])
                        zas[i], zbs[i] = za, zb
                for i in blk:
                    st = (i == 0)
                    nc.tensor.matmul(accA[:, nsl], wa[:, i * S:i * S + 128],
                                     zas[i][:, nsl], start=st, stop=False)
                    nc.tensor.matmul(accA[:, nsl], wb[:, i * S:i * S + 128],
                                     zbs[i][:, nsl], start=False,
                                     stop=(i == F0 - 1))
                for i in blk:
                    st = (i == 0)
                    sp = (i == F0 - 1)
                    nc.tensor.matmul(accB[:, nsl],
                                     wa[:, i * S + 128:(i + 1) * S],
                                     zas[i][:, nsl], start=st, stop=False)
                    nc.tensor.matmul(accB[:, nsl],
                                     wb[:, i * S + 128:(i + 1) * S],
                                     zbs[i][:, nsl], start=False, stop=sp)
                if feeds:
                    for _ in range(3):
                        if feeds:
                            feeds.pop(0)()
            while feeds:
                feeds.pop(0)()
            st8[ct]["acc"] = (accA, accB)

        # ---- software-pipelined emission over column tiles ----
        # While ct runs layers 1-2 on the PE, ct+1's broadcast DMAs and
        # layer 0 fill the PE/DVE gaps (acc bufs=3 makes a third accumulation
        # group available).
        emit_xf(0)
        for k in range(L0K):
            l0_feed(0, k)
        emit_l0(0, first=True)
        for ct in range(NCT):
            hA, hB, red0 = drain_acc(ct, 0, defer=True)
            if ct + 1 < NCT:
                emit_xf(ct + 1)
                feeds = [red0] + \
                    [(lambda cc, kk: lambda: l0_feed(cc, kk))(ct + 1, k)
                     for k in range(L0K)]
            else:
                feeds = [red0]
            emit_layer(ct, 1, w1a, w1b, hA, hB, feeds=feeds)
            if ct + 1 < NCT:
                emit_l0(ct + 1)
            hA, hB, red1 = drain_acc(ct, 1, defer=True)
            emit_layer(ct, 2, w2a, w2b, hA, hB, feeds=[red1])
            drain_acc(ct, 2)
            st8.pop(ct - 1, None)

        # ---- transpose [n, b] -> [b, n] and assemble final [64, 600] ----
        for l in range(3):
            tpA = ps.tile([BPC, 128], dt.float32, name=f"tpA{l}", tag="tps")
            nc.tensor.transpose(tpA[:], outA[l][:], ident[:])
            nc.scalar.copy(final[:, l * S:l * S + 128], tpA[:])
            tpB = ps.tile([BPC, 72], dt.float32, name=f"tpB{l}", tag="tps")
            nc.tensor.transpose(tpB[:], outB[l][:], ident[0:72, 0:72])
            nc.scalar.copy(final[:, l * S + 128:(l + 1) * S], tpB[:])

        nc.sync.dma_start(y_d[:], final[:])

        if rep_ctx is not None:
            rep_ctx.__exit__(None, None, None)

    nc.compile()
    return nc


def _prep_consts(W0, W1, W2):
    """Host-side constant tensors shared by all cores (bf16)."""
    W0b = W0.astype(bf16)
    W1b = W1.astype(bf16)
    W2b = W2.astype(bf16)

    # layer-0 weights: chunks of 120 interaction rows, zero-padded to 14*120
    w0p = np.zeros((120, L0K * S), dtype=bf16)
    for k in range(L0K):
        r0 = k * 120
        rows = min(120, F0 * F0 - r0)
        w0p[:rows, k * S:(k + 1) * S] = W0b[r0:r0 + rows]

    # layer 1/2 weights: per-i chunks, j 0..127 (A) and j 128..199 (B)
    def wsplit(Wb):
        wa = np.zeros((128, F0 * S), dtype=bf16)
        wb = np.zeros((72, F0 * S), dtype=bf16)
        for i in range(F0):
            wa[:, i * S:(i + 1) * S] = Wb[i * S:i * S + 128]
            wb[:, i * S:(i + 1) * S] = Wb[i * S + 128:(i + 1) * S]
        return wa, wb

    w1a, w1b = wsplit(W1b)
    w2a, w2b = wsplit(W2b)

    # selector for broadcasting x0T row i across 128 partitions
    sel = np.zeros((F0, F0 * 128), dtype=bf16)
    for i in range(F0):
        sel[i, i * 128:(i + 1) * 128] = 1.0

    # selector for the layer-0 x0 broadcast pattern (3 i-blocks per chunk)
    t0sel = np.zeros((F0, L0K * 120), dtype=bf16)
    for k in range(L0K):
        for r in range(120):
            i = 3 * k + r // F0
            if i < F0:
                t0sel[i, k * 120 + r] = 1.0

    ident = np.eye(128, dtype=np.float32)
    return dict(w0=w0p, w1a=w1a, w1b=w1b, w2a=w2a, w2b=w2b, ident=ident,
                sel=sel, t0sel=t0sel)


def _prep_in_maps(inputs, W0, W1, W2):
    consts = _prep_consts(np.asarray(W0, np.float32),
                          np.asarray(W1, np.float32),
                          np.asarray(W2, np.float32))
    x = np.asarray(inputs, np.float32)
    if XMODE == "dma":
        consts.pop("sel", None)
    in_maps = []
    for c in range(NCORES):
        xs = x[c * BPC:(c + 1) * BPC]                    # [64, 40, 32]
        x0T = np.ascontiguousarray(
            xs.transpose(1, 0, 2).reshape(F0, C)).astype(bf16)
        x0T3 = np.ascontiguousarray(np.tile(x0T, (3, 1)))  # [120, C]
        in_maps.append(dict(x0T=x0T, x0T3=x0T3, **consts))
    return in_maps


def _get_nc():
    if "nc" not in _CACHE:
        _CACHE["nc"] = _build_nc()
    return _CACHE["nc"]


def kernel(inputs, W0, W1, W2):
    from concourse.bass_utils import run_bass_kernel_spmd

    nc = _get_nc()
    in_maps = _prep_in_maps(inputs, W0, W1, W2)
    res = run_bass_kernel_spmd(nc, in_maps, core_ids=list(range(NCORES)))
    y = np.concatenate([res.results[c]["y"] for c in range(NCORES)], axis=0)
    return np.ascontiguousarray(y, dtype=np.float32)


# revision 42
# speedup vs baseline: 4008.2606x; 1.7538x over previous
# CIN (Compressed Interaction Network) Trainium2 Bass kernel.
#
# Reference computation (B=512, F0=40, D=32, sizes=[200,200,200]):
#   h0 = x                                  [B, 40, 32]
#   per layer l: z[b,(i,j),d] = x[b,i,d] * h[b,j,d];  h' = z^T W_l  [B, 200, 32]
#   out = concat(h1,h2,h3, axis=1).sum(-1)  [B, 600]
#
# Strategy: data-parallel over batch across 8 cores (64 batches/core).
# Per core, columns c = b_local*32 + d (C=2048), processed in 4 tiles of 512.
# All tensors are kept in a [row, c] layout so each layer's GEMM output
# (h'[n, c]) is directly consumable as the next layer's h[j, c] input:
#   zT[(i,j), c] = x0T[i, c] * hT[j, c]
#   h'[n, c]     = sum_k W[k, n] * zT[k, c]   (PE, contraction over partitions)
# The x0T[i,:] partition-broadcast tiles are produced by K=40 selector
# matmuls on the PE (one per i), converted to bf16 by the scalar engine, and
# the elementwise multiply runs on the vector engine in bf16 (2x mode).
# Contraction is chunked per-i as K=128 (j 0..127) + K=72 (j 128..199) so the
# vector-engine operands stay partition-aligned with the hT tiles.

import dataclasses

import numpy as np
import ml_dtypes

B, F0, D, S = 512, 40, 32, 200
NCORES = 8
BPC = B // NCORES          # 64 batches per core
C = BPC * D                # 2048 columns per core
CT = 512                   # column tile
NCT = C // CT              # 4 column tiles
L0K = 14                   # layer-0 chunks of 120 rows (last chunk zero-padded)

bf16 = ml_dtypes.bfloat16

# production configuration
XMODE, GB, TTW = "dma", 8, 4

_CACHE = {}


def _build_nc(repeat=1, skip=(), xmode=XMODE, gb=GB, ttw=TTW):
    """skip: ablation variants for timing. xmode: "pe" (selector matmuls) or
    "dma" (doubling chains) for the x0 broadcast tiles. gb: i-block size for
    PSUM-bank-grouped matmul emission."""
    from contextlib import ExitStack
    import concourse.tile as tile
    from concourse import bacc, mybir

    nsl = slice(0, 4) if "mm" in skip else slice(0, None)

    nc = bacc.Bacc("TRN2", target_bir_lowering=False, debug=False,
                   num_devices=NCORES)

    dt = mybir.dt
    ein, eout = "ExternalInput", "ExternalOutput"
    x0T_d = nc.dram_tensor("x0T", [F0, C], dt.bfloat16, kind=ein).ap()
    x0T3_d = nc.dram_tensor("x0T3", [120, C], dt.bfloat16, kind=ein).ap()
    w0_d = nc.dram_tensor("w0", [120, L0K * S], dt.bfloat16, kind=ein).ap()
    t0sel_d = nc.dram_tensor("t0sel", [F0, L0K * 120], dt.bfloat16,
                             kind=ein).ap()
    if xmode == "pe":
        sel_d = nc.dram_tensor("sel", [F0, F0 * 128], dt.bfloat16,
                               kind=ein).ap()
    w1a_d = nc.dram_tensor("w1a", [128, F0 * S], dt.bfloat16, kind=ein).ap()
    w1b_d = nc.dram_tensor("w1b", [72, F0 * S], dt.bfloat16, kind=ein).ap()
    w2a_d = nc.dram_tensor("w2a", [128, F0 * S], dt.bfloat16, kind=ein).ap()
    w2b_d = nc.dram_tensor("w2b", [72, F0 * S], dt.bfloat16, kind=ein).ap()
    ident_d = nc.dram_tensor("ident", [128, 128], dt.float32, kind=ein).ap()
    y_d = nc.dram_tensor("y", [BPC, 3 * S], dt.float32, kind=eout).ap()

    with tile.TileContext(nc) as tc, ExitStack() as ctx:
        const = ctx.enter_context(tc.tile_pool(name="const", bufs=1))
        xpool = ctx.enter_context(tc.tile_pool(name="xpool", bufs=1))
        zpool = ctx.enter_context(tc.tile_pool(name="zpool", bufs=4))
        hpool = ctx.enter_context(tc.tile_pool(name="hpool", bufs=2))
        ps = ctx.enter_context(tc.tile_pool(name="ps", bufs=2, space="PSUM"))

        def const_tile(name, shape, dtp, src):
            t = const.tile(shape, dtp, name=name, tag=name)
            nc.sync.dma_start(t[:], src[:])
            return t

        x0T = const_tile("x0T_sb", [F0, C], dt.bfloat16, x0T_d)
        x0T3 = const_tile("x0T3_sb", [120, C], dt.bfloat16, x0T3_d)
        w0 = const_tile("w0_sb", [120, L0K * S], dt.bfloat16, w0_d)
        t0sel = const_tile("t0sel_sb", [F0, L0K * 120], dt.bfloat16, t0sel_d)
        if xmode == "pe":
            sel = const_tile("sel_sb", [F0, F0 * 128], dt.bfloat16, sel_d)
        # w1/w2 tiles are allocated now but their load DMAs are issued after
        # the first layer-0 chunk so they don't head-block the HWDGE ring
        w1a = const.tile([128, F0 * S], dt.bfloat16, name="w1a_sb", tag="w1a_sb")
        w1b = const.tile([72, F0 * S], dt.bfloat16, name="w1b_sb", tag="w1b_sb")
        w2a = const.tile([128, F0 * S], dt.bfloat16, name="w2a_sb", tag="w2a_sb")
        w2b = const.tile([72, F0 * S], dt.bfloat16, name="w2b_sb", tag="w2b_sb")
        ident = const_tile("ident_sb", [128, 128], dt.float32, ident_d)
        wloads = [(w1a, w1a_d), (w1b, w1b_d), (w2a, w2a_d), (w2b, w2b_d)]

        # per-layer output accumulators [n, b_local] in fp32
        outA = [const.tile([128, BPC], dt.float32, name=f"outA{l}", tag=f"outA{l}")
                for l in range(3)]
        outB = [const.tile([72, BPC], dt.float32, name=f"outB{l}", tag=f"outB{l}")
                for l in range(3)]
        final = const.tile([BPC, 3 * S], dt.float32, name="final", tag="final")

        rep_ctx = tc.For_i(0, repeat, 1) if repeat > 1 else None
        if rep_ctx is not None:
            rep_ctx.__enter__()

        HI = F0 // 2
        zsl = slice(0, 4) if "tt" in skip else slice(0, CT)
        st8 = {}   # per-ct pipeline state: xfs, xts, acc

        def emit_xf(ct):
            """x0 broadcast tensors Xf[p, i*CT+c] = x0T[i, ct*CT+c] (dma)."""
            c0 = ct * CT
            csl = slice(c0, c0 + CT)
            xfs = []
            if xmode == "dma":
                for h in range(2):
                    # both chains on the SP ring: ACT only carries the x0p
                    # copies, which feed layer 0 and must not queue behind
                    # chain semaphore waits
                    eng = nc.sync
                    xf = xpool.tile([128, HI * CT], dt.bfloat16,
                                    name=f"xf{h}_{ct}", tag=f"xf{h}", bufs=2)
                    if "xf" in skip:
                        eng.dma_start(
                            xf[0:1, 0:HI].rearrange("p (i c) -> p i c", c=1),
                            x0T_d[h * HI:(h + 1) * HI, c0:c0 + 1])
                    else:
                        # seed rows 0..7 independently from DRAM (no chain
                        # deps), then 4 row-doubling DMAs: 8->16->32->64->128
                        for r in range(8):
                            eng.dma_start(
                                xf[r:r + 1, :].rearrange(
                                    "p (i c) -> p i c", c=CT),
                                x0T_d[h * HI:(h + 1) * HI, csl])
                        n = 8
                        while n < 128:
                            eng.dma_start(xf[n:2 * n, :], xf[0:n, :])
                            n *= 2
                    xfs.append(xf)
            st8[ct] = {"xfs": xfs, "xts": {}}

        def make_xt(ct, i):
            # X_i[p, c] = x0T[i, c] via K=40 selector matmul + ACT copy
            c0 = ct * CT
            xp = ps.tile([128, CT], dt.float32, name=f"xp{i}", tag="xps")
            nc.tensor.matmul(xp[:], sel[:, i * 128:(i + 1) * 128],
                             x0T[:, c0:c0 + CT], start=True, stop=True)
            xi = xpool.tile([128, CT], dt.bfloat16, name=f"x{i}", tag=f"x{i}")
            nc.scalar.copy(xi[:], xp[:])
            st8[ct]["xts"][i] = xi

        def xslice(ct, i, rows=slice(0, 128)):
            if xmode == "pe":
                return st8[ct]["xts"][i][rows, :]
            h, ii = divmod(i, HI)
            return st8[ct]["xfs"][h][rows, ii * CT:(ii + 1) * CT]

        def l0_feed(ct, k):
            """x0p broadcast pattern + z0 TT for layer-0 chunk k."""
            c0 = ct * CT
            csl = slice(c0, c0 + CT)
            x0p = xpool.tile([120, CT], dt.bfloat16, name=f"x0p{k}",
                             tag="x0p", bufs=4)
            # selector matmul feed: no DMA dependencies, pipelines PE->ACT->
            # DVE tightly (the PE cost is ~3us/ct, ACT is otherwise idle)
            xp0 = ps.tile([128, CT], dt.float32, name=f"xp0{k}", tag="tps")
            nc.tensor.matmul(xp0[0:120, :],
                             t0sel[:, k * 120:(k + 1) * 120],
                             x0T[:, csl], start=True, stop=True)
            nc.scalar.copy(x0p[:], xp0[0:120, :])
            z0 = zpool.tile([120, CT], dt.bfloat16, name=f"z0_{ct}_{k}",
                            tag="z0", bufs=11)
            nc.vector.tensor_mul(z0[:, zsl], x0T3[:, c0:c0 + zsl.stop],
                                 x0p[:, zsl])
            st8[ct].setdefault("z0s", {})[k] = z0

        def emit_l0(ct, first=False):
            """Layer 0 GEMM (feeds must have been emitted)."""
            nonlocal wloads
            accA = ps.tile([128, CT], dt.float32, name=f"l0A{ct}", tag="accA",
                           bufs=3)
            accB = ps.tile([72, CT], dt.float32, name=f"l0B{ct}", tag="accB",
                           bufs=3)
            z0s = st8[ct]["z0s"]
            for k in range(L0K):
                if k not in z0s:
                    l0_feed(ct, k)
                z0 = z0s[k]
                nc.tensor.matmul(accA[:, nsl], w0[:, k * S:k * S + 128],
                                 z0[:, nsl],
                                 start=(k == 0), stop=(k == L0K - 1))
                nc.tensor.matmul(accB[:, nsl], w0[:, k * S + 128:(k + 1) * S],
                                 z0[:, nsl],
                                 start=(k == 0), stop=(k == L0K - 1))
                if first and k in (4, 10) and wloads:
                    # stagger the big weight loads so they don't contend with
                    # the x0 broadcast chains on the DMA engines at startup
                    for wt, wd in wloads[:2]:
                        nc.sync.dma_start(wt[:], wd[:])
                    wloads = wloads[2:]
            st8[ct]["acc"] = (accA, accB)

        def bcast4(ap):
            # free-dim step-0 broadcast: [p, CT] read as [p, 4, CT]
            a = [list(d) for d in ap.ap]
            return dataclasses.replace(ap, ap=[a[0], [0, 4], a[1]])

        def drain_acc(ct, l, defer=False):
            """Produce the bf16 h tiles for the next layer (critical path),
            returning the output d-sum reduces as deferred thunks so they
            don't sit between the h copies and the next layer's first TT in
            the DVE stream."""
            accA, accB = st8[ct]["acc"]
            hA = hB = None
            if l < 2:
                hA = hpool.tile([128, CT], dt.bfloat16, name=f"hA{l}",
                                tag="hA")
                hB = hpool.tile([72, CT], dt.bfloat16, name=f"hB{l}",
                                tag="hB")
                nc.vector.tensor_copy(hA[:], accA[:])
                nc.vector.tensor_copy(hB[:], accB[:])

            def reduces():
                nc.vector.tensor_reduce(
                    outA[l][:, ct * 16:(ct + 1) * 16],
                    accA[:].rearrange("p (b d) -> p b d", d=D),
                    axis=mybir.AxisListType.X, op=mybir.AluOpType.add)
                nc.vector.tensor_reduce(
                    outB[l][:, ct * 16:(ct + 1) * 16],
                    accB[:].rearrange("p (b d) -> p b d", d=D),
                    axis=mybir.AxisListType.X, op=mybir.AluOpType.add)

            if defer:
                return hA, hB, reduces
            reduces()
            return hA, hB

        def emit_layer(ct, l, wa, wb, hA, hB, feeds=None):
            """One interaction layer: z TTs + per-i GEMM accumulation.
            feeds: optional list of thunks (next-ct layer-0 feed work) to
            interleave between i-blocks so the DVE stream stays ahead."""
            xfs = st8[ct]["xfs"]
            accA = ps.tile([128, CT], dt.float32, name=f"l{l}A{ct}",
                           tag="accA", bufs=3)
            accB = ps.tile([72, CT], dt.float32, name=f"l{l}B{ct}",
                           tag="accB", bufs=3)
            for i0 in range(0, F0, gb):
                blk = range(i0, min(i0 + gb, F0))
                zas, zbs = {}, {}
                if ttw == 4 and xmode == "dma" and "tt" not in skip:
                    for j0 in range(i0, min(i0 + gb, F0), 4):
                        h4, ii = divmod(j0, HI)
                        x4 = xfs[h4][:, ii * CT:(ii + 4) * CT]
                        za4 = zpool.tile([128, 4 * CT], dt.bfloat16,
                                         name=f"za{l}_{j0}", tag="za",
                                         bufs=3)
                        nc.vector.tensor_mul(
                            za4[:].rearrange("p (r c) -> p r c", r=4),
                            bcast4(hA[:]),
                            x4.rearrange("p (r c) -> p r c", r=4))
                        zb4 = zpool.tile([72, 4 * CT], dt.bfloat16,
                                         name=f"zb{l}_{j0}", tag="zb",
                                         bufs=3)
                        nc.vector.tensor_mul(
                            zb4[:].rearrange("p (r c) -> p r c", r=4),
                            bcast4(hB[:]),
                            x4[0:72, :].rearrange("p (r c) -> p r c", r=4))
                        for i in range(j0, j0 + 4):
                            o = (i - j0) * CT
                            zas[i] = za4[:, o:o + CT]
                            zbs[i] = zb4[:, o:o + CT]
                else:
                    for i in blk:
                        if xmode == "pe" and l == 1:
                            make_xt(ct, i)
                        za = zpool.tile([128, CT], dt.bfloat16,
                                        name=f"za{l}_{i}", tag="za",
                                        bufs=gb + 2)
                        nc.vector.tensor_mul(za[:, zsl], hA[:, zsl],
                                             xslice(ct, i)[:, zsl])
                        zb = zpool.tile([72, CT], dt.bfloat16,
                                        name=f"zb{l}_{i}", tag="zb",
                                        bufs=gb + 2)
                        nc.vector.tensor_mul(zb[:, zsl], hB[:, zsl],
                                             xslice(ct, i, slice(0, 72))[:, zsl])
                        zas[i], zbs[i] = za, zb
                for i in blk:
                    st = (i == 0)
                    nc.tensor.matmul(accA[:, nsl], wa[:, i * S:i * S + 128],
                                     zas[i][:, nsl], start=st, stop=False)
                    nc.tensor.matmul(accA[:, nsl], wb[:, i * S:i * S + 128],
                                     zbs[i][:, nsl], start=False,
                                     stop=(i == F0 - 1))
                for i in blk:
                    st = (i == 0)
                    sp = (i == F0 - 1)
                    nc.tensor.matmul(accB[:, nsl],
                                     wa[:, i * S + 128:(i + 1) * S],
                                     zas[i][:, nsl], start=st, stop=False)
                    nc.tensor.matmul(accB[:, nsl],
                                     wb[:, i * S + 128:(i + 1) * S],
                                     zbs[i][:, nsl], start=False, stop=sp)
                if feeds:
                    for _ in range(3):
                        if feeds:
                            feeds.pop(0)()
            while feeds:
                feeds.pop(0)()
            st8[ct]["acc"] = (accA, accB)

        # ---- software-pipelined emission over column tiles ----
        # While ct runs layers 1-2 on the PE, ct+1's broadcast DMAs and
        # layer 0 fill the PE/DVE gaps (acc bufs=3 makes a third accumulation
        # group available).
        emit_xf(0)
        for k in range(L0K):
            l0_feed(0, k)
        emit_l0(0, first=True)
        for ct in range(NCT):
            hA, hB, red0 = drain_acc(ct, 0, defer=True)
            if ct + 1 < NCT:
                emit_xf(ct + 1)
                feeds = [red0] + \
                    [(lambda cc, kk: lambda: l0_feed(cc, kk))(ct + 1, k)
                     for k in range(L0K)]
            else:
                feeds = [red0]
            emit_layer(ct, 1, w1a, w1b, hA, hB, feeds=feeds)
            if ct + 1 < NCT:
                emit_l0(ct + 1)
            hA, hB, red1 = drain_acc(ct, 1, defer=True)
            emit_layer(ct, 2, w2a, w2b, hA, hB, feeds=[red1])
            drain_acc(ct, 2)
            st8.pop(ct - 1, None)

        # ---- transpose [n, b] -> [b, n] and assemble final [64, 600] ----
        for l in range(3):
            tpA = ps.tile([BPC, 128], dt.float32, name=f"tpA{l}", tag="tps")
            nc.tensor.transpose(tpA[:], outA[l][:], ident[:])
            nc.scalar.copy(final[:, l * S:l * S + 128], tpA[:])
            tpB = ps.tile([BPC, 72], dt.float32, name=f"tpB{l}", tag="tps")
            nc.tensor.transpose(tpB[:], outB[l][:], ident[0:72, 0:72])
            nc.scalar.copy(final[:, l * S + 128:(l + 1) * S], tpB[:])

        nc.sync.dma_start(y_d[:], final[:])

        if rep_ctx is not None:
            rep_ctx.__exit__(None, None, None)

    nc.compile()
    return nc


def _prep_consts(W0, W1, W2):
    """Host-side constant tensors shared by all cores (bf16)."""
    W0b = W0.astype(bf16)
    W1b = W1.astype(bf16)
    W2b = W2.astype(bf16)

    # layer-0 weights: chunks of 120 interaction rows, zero-padded to 14*120
    w0p = np.zeros((120, L0K * S), dtype=bf16)
    for k in range(L0K):
        r0 = k * 120
        rows = min(120, F0 * F0 - r0)
        w0p[:rows, k * S:(k + 1) * S] = W0b[r0:r0 + rows]

    # layer 1/2 weights: per-i chunks, j 0..127 (A) and j 128..199 (B)
    def wsplit(Wb):
        wa = np.zeros((128, F0 * S), dtype=bf16)
        wb = np.zeros((72, F0 * S), dtype=bf16)
        for i in range(F0):
            wa[:, i * S:(i + 1) * S] = Wb[i * S:i * S + 128]
            wb[:, i * S:(i + 1) * S] = Wb[i * S + 128:(i + 1) * S]
        return wa, wb

    w1a, w1b = wsplit(W1b)
    w2a, w2b = wsplit(W2b)

    # selector for broadcasting x0T row i across 128 partitions
    sel = np.zeros((F0, F0 * 128), dtype=bf16)
    for i in range(F0):
        sel[i, i * 128:(i + 1) * 128] = 1.0

    # selector for the layer-0 x0 broadcast pattern (3 i-blocks per chunk)
    t0sel = np.zeros((F0, L0K * 120), dtype=bf16)
    for k in range(L0K):
        for r in range(120):
            i = 3 * k + r // F0
            if i < F0:
                t0sel[i, k * 120 + r] = 1.0

    ident = np.eye(128, dtype=np.float32)
    return dict(w0=w0p, w1a=w1a, w1b=w1b, w2a=w2a, w2b=w2b, ident=ident,
                sel=sel, t0sel=t0sel)


def _prep_in_maps(inputs, W0, W1, W2):
    consts = _prep_consts(np.asarray(W0, np.float32),
                          np.asarray(W1, np.float32),
                          np.asarray(W2, np.float32))
    x = np.asarray(inputs, np.float32)
    if XMODE == "dma":
        consts.pop("sel", None)
    in_maps = []
    for c in range(NCORES):
        xs = x[c * BPC:(c + 1) * BPC]                    # [64, 40, 32]
        x0T = np.ascontiguousarray(
            xs.transpose(1, 0, 2).reshape(F0, C)).astype(bf16)
        x0T3 = np.ascontiguousarray(np.tile(x0T, (3, 1)))  # [120, C]
        in_maps.append(dict(x0T=x0T, x0T3=x0T3, **consts))
    return in_maps


def _get_nc():
    if "nc" not in _CACHE:
        _CACHE["nc"] = _build_nc()
    return _CACHE["nc"]


def kernel(inputs, W0, W1, W2):
    from concourse.bass_utils import run_bass_kernel_spmd

    nc = _get_nc()
    in_maps = _prep_in_maps(inputs, W0, W1, W2)
    res = run_bass_kernel_spmd(nc, in_maps, core_ids=list(range(NCORES)))
    y = np.concatenate([res.results[c]["y"] for c in range(NCORES)], axis=0)
    return np.ascontiguousarray(y, dtype=np.float32)
